# revision 1
# baseline (speedup 1.0000x reference)
"""Bass/Trainium2 kernel for BiGraphContrastLayer (GNN message passing).

Computes, for two edge lists (pos/neg) over the same node features:
    h_g = PReLU( D_in^-1/2 A_g D_out^-1/2 feats @ W + b )
returning stack([h_pos, h_neg]) of shape [2, N, Dout].

Strategy (8 NeuronCores, SPMD, no collectives), using the linearity
   (D_in^-1/2 A D_out^-1/2 feats) @ W = D_in^-1/2 A (D_out^-1/2 feats W):

  Phase 1 (y-phase): y_g = bf16( ns_g * (feats @ W) ), computed from a
    host-transposed feats (so featsT tiles are directly the matmul lhsT),
    with the per-node ns scale applied on the PSUM->SBUF read. Stored to
    DRAM per graph.
  Phase 2 (gather/aggregate): edges are bucketed by 128-node dst tile and
    sorted by src. dma_gather (int16, 4 row-banks of <=25088 rows) pulls
    y[src] rows for ~128-edge chunks; a one-hot matmul (lhsT = onehot of
    dst offsets) segment-sums each chunk into the dst tile's PSUM
    accumulator. Final nd-scale + PReLU on DVE, store.

  Host does integer index work only: degree bincounts, sorting, bucketing,
  dealing dst tiles to cores so all 8 cores share one instruction stream
  (signature-matched by per-bank chunk counts), building int16 wrapped
  gather indices, and replicating the small params per the sharding hint.
"""

import math
import tempfile
from dataclasses import dataclass

import numpy as np

P = 128   # partitions
D = 128   # feature dim (Din == Dout == 128)
NBANK = 4


# --------------------------------------------------------------------------
# Config
# --------------------------------------------------------------------------
@dataclass
class Config:
    n_nodes: int = 100000
    n_cores: int = 8
    xbatch: int = 8    # feats tiles per y-phase batch
    sg: int = 10       # dst-tile positions per gather supergroup
    y_act_split: bool = True  # pos-graph y scale on ScalarE, neg on DVE
    act_prelu: bool = True    # final nd-scale+PReLU on ScalarE (not in sim)
    oh_gpsimd_mod: int = 0    # every Nth one-hot build goes to GpSimd (0=off)
    gbufs: int = 2            # gather buffer count

    @property
    def t_global(self) -> int:
        return math.ceil(self.n_nodes / P)

    @property
    def n_pad(self) -> int:
        return self.t_global * P

    @property
    def t_core(self) -> int:
        return math.ceil(self.t_global / self.n_cores)

    @property
    def bank_tiles(self) -> int:
        return math.ceil(self.t_global / NBANK)

    @property
    def bank_rows(self) -> int:
        return self.bank_tiles * P


# --------------------------------------------------------------------------
# Host-side preprocessing (integer index manipulation only)
# --------------------------------------------------------------------------
def _row_of(n, cfg: Config):
    """y DRAM row of node n: within each xbatch of tiles, rows are laid
    p-major (node (t0+i)*128+p -> row t0*128 + p*nb + i) so the y-phase
    store writes nb*256B contiguous per partition."""
    xb, tg = cfg.xbatch, cfg.t_global
    t = n // P
    p = n % P
    t0 = (t // xb) * xb
    nb = np.minimum(xb, tg - t0)
    return t0 * P + p * nb + (t - t0)


def _plan_graph(src, dst, cfg: Config):
    """Bucket edges by dst tile, sort by src within tile, bank-split, and
    deal tiles to cores with per-bank-signature matching.

    Returns dict with:
      core_tiles  [n_cores, t_core]  global tile id per position (-1 null)
      cstar       [t_core, NBANK]    shared chunk counts per position/bank
      tile_edges  list per global tile: (src_sorted, off_sorted, bank_cnt)
    """
    tg, ncores, tcore = cfg.t_global, cfg.n_cores, cfg.t_core
    order = np.argsort(dst, kind="stable")
    src_s = src[order]
    dst_s = dst[order]
    tile_cnt = np.bincount(dst_s // P, minlength=tg)
    starts = np.zeros(tg + 1, np.int64)
    np.cumsum(tile_cnt, out=starts[1:])

    row_s = _row_of(src_s, cfg)
    bank_of = row_s // cfg.bank_rows
    sig = np.zeros((tg, NBANK), np.int64)
    tile_edges = []
    for t in range(tg):
        e0, e1 = int(starts[t]), int(starts[t + 1])
        so = np.argsort(row_s[e0:e1], kind="stable")
        ts_src = row_s[e0:e1][so]
        ts_off = (dst_s[e0:e1][so] % P).astype(np.int64)
        bc = np.bincount(bank_of[e0:e1], minlength=NBANK)
        sig[t] = -(-bc // P)  # ceil chunks per bank
        sig[t, 0] = max(sig[t, 0], 1)
        tile_edges.append((ts_src, ts_off, bc))

    # Deal: sort tiles by signature so consecutive groups of n_cores tiles
    # have matching/near-matching per-bank chunk counts.
    sigkey = sig @ (np.int64(32) ** np.arange(NBANK - 1, -1, -1))
    keys = np.argsort(sigkey, kind="stable")
    # pad with null tiles to n_cores * t_core
    n_slots = ncores * tcore
    dealt = np.full(n_slots, -1, np.int64)
    dealt[: len(keys)] = keys[::-1]  # descending signature order
    core_tiles = np.zeros((ncores, tcore), np.int64)
    cstar = np.zeros((tcore, NBANK), np.int64)
    for k in range(tcore):
        grp = dealt[k * ncores : (k + 1) * ncores]
        for c in range(ncores):
            core_tiles[c, k] = grp[c]
        s = np.zeros(NBANK, np.int64)
        for t in grp:
            if t >= 0:
                s = np.maximum(s, sig[t])
        s[0] = max(s[0], 1)
        cstar[k] = s
    return dict(core_tiles=core_tiles, cstar=cstar, tile_edges=tile_edges)


def _slot_layout(cstar, cfg: Config):
    """Shared (all-cores) slot layout for one graph.

    Slot space = sequence of supergroups; within a supergroup, bank-major:
      for b in banks: for k in sg positions: cstar[k, b] chunks.
    Returns:
      sg_list: list of (k0, kn)
      calls:   list of (sg_idx, bank, chunk0, nchunks)  [nchunks > 0]
      chunk_of: dict (k, b, c) -> global chunk index
      n_chunks total
    """
    tcore, sg = cfg.t_core, cfg.sg
    sg_list = []
    k0 = 0
    while k0 < tcore:
        kn = min(sg, tcore - k0)
        sg_list.append((k0, kn))
        k0 += kn
    calls = []
    chunk_of = {}
    cpos = 0
    for si, (k0, kn) in enumerate(sg_list):
        for b in range(NBANK):
            c0 = cpos
            for ki in range(kn):
                k = k0 + ki
                for c in range(int(cstar[k, b])):
                    chunk_of[(k, b, c)] = cpos
                    cpos += 1
            if cpos > c0:
                calls.append((si, b, c0, cpos - c0))
    return sg_list, calls, chunk_of, cpos


def _fill_core_graph(plan, layout, core, cfg: Config):
    """Build IDX16 (wrapped) and OFF arrays for one core, one graph."""
    sg_list, calls, chunk_of, n_chunks = layout
    cstar = plan["cstar"]
    idx = np.zeros((n_chunks, P), np.int16)
    off = np.full((n_chunks, P), 512.0, np.float32)
    for k in range(cfg.t_core):
        t = plan["core_tiles"][core, k]
        if t < 0:
            continue
        ts_src, ts_off, bc = plan["tile_edges"][t]
        bstart = np.zeros(NBANK + 1, np.int64)
        np.cumsum(bc, out=bstart[1:])
        for b in range(NBANK):
            nb = int(bc[b])
            cnum = int(cstar[k, b])
            if cnum == 0:
                continue
            nslot = cnum * P
            es = np.zeros(nslot, np.int64)
            eo = np.full(nslot, 512.0, np.float32)
            es[:nb] = ts_src[bstart[b] : bstart[b + 1]] - b * cfg.bank_rows
            eo[:nb] = ts_off[bstart[b] : bstart[b + 1]]
            for c in range(cnum):
                ci = chunk_of[(k, b, c)]
                idx[ci] = es[c * P : (c + 1) * P]
                off[ci] = eo[c * P : (c + 1) * P]
    # wrap: flat slot i (within a call's range) -> [i%16, i//16], replicated
    # to 128 partitions. Build per call, concatenated along columns.
    ncols = n_chunks * P // 16
    idx_w = np.zeros((P, ncols), np.int16)
    for (si, b, c0, nch) in calls:
        flat = idx[c0 : c0 + nch].reshape(-1)  # ni slots
        blk = flat.reshape(-1, 16).T  # [16, ni/16]
        idx_w[:, c0 * 8 : (c0 + nch) * 8] = np.tile(blk, (8, 1))
    return idx_w, off.T.copy()  # off -> [P, n_chunks] f32


def preprocess(feats, W, b, prelu_a, src_pos, dst_pos, src_neg, dst_neg,
               cfg: Config):
    n, ncores, tcore, tg = cfg.n_nodes, cfg.n_cores, cfg.t_core, cfg.t_global
    feats = np.asarray(feats, np.float32)
    W = np.asarray(W, np.float32)
    b = np.asarray(b, np.float32)
    prelu_a = np.asarray(prelu_a, np.float32)

    feats_pad = np.zeros((cfg.n_pad, D), np.float32)
    feats_pad[:n] = feats
    featsT = np.ascontiguousarray(feats_pad.T)  # [D, n_pad]

    plans, layouts, deg_outs, deg_ins = [], [], [], []
    for src, dst in ((src_pos, dst_pos), (src_neg, dst_neg)):
        src = np.asarray(src, np.int64)
        dst = np.asarray(dst, np.int64)
        deg_outs.append(np.bincount(src, minlength=n).astype(np.int32))
        deg_ins.append(np.bincount(dst, minlength=n).astype(np.int32))
        plan = _plan_graph(src, dst, cfg)
        plans.append(plan)
        layouts.append(_slot_layout(plan["cstar"], cfg))

    dego_arr = np.zeros((P, 2 * tg), np.int32)
    for g in range(2):
        dpad = np.zeros(cfg.n_pad, np.int32)
        dpad[:n] = deg_outs[g]
        dego_arr[:, g * tg : (g + 1) * tg] = dpad.reshape(tg, P).T

    degi_arr = np.zeros((ncores, P, 2 * tcore), np.int32)
    for g in range(2):
        dpad = np.zeros(cfg.n_pad, np.int32)
        dpad[:n] = deg_ins[g]
        dpad_t = dpad.reshape(tg, P).T
        for core in range(ncores):
            ct = plans[g]["core_tiles"][core]
            valid = ct >= 0
            degi_arr[core, :, g * tcore : (g + 1) * tcore][:, valid] = (
                dpad_t[:, ct[valid]])

    iota = np.tile(np.arange(P, dtype=np.float32), (P, 1)).astype(
        np.dtype("bfloat16"))
    a_rep = np.full((P, 1), float(prelu_a.reshape(-1)[0]), np.float32)
    b_rep = np.tile(b.reshape(1, D), (P, 1)).astype(np.float32)

    in_maps = []
    for core in range(ncores):
        iw_p, off_p = _fill_core_graph(plans[0], layouts[0], core, cfg)
        iw_n, off_n = _fill_core_graph(plans[1], layouts[1], core, cfg)
        in_maps.append({
            "featst": featsT,
            "w_in": W,
            "a_rep": a_rep,
            "b_rep": b_rep,
            "dego": dego_arr,
            "degi": degi_arr[core],
            "idx_in": np.concatenate([iw_p, iw_n], axis=1),
            "off_in": np.concatenate([off_p, off_n], axis=1),
            "iota_in": iota,
        })
    meta = {
        "layouts": layouts,
        "cstar": [plans[0]["cstar"], plans[1]["cstar"]],
        "use_bias": bool(np.any(b != 0.0)),
    }
    return in_maps, plans, meta


# --------------------------------------------------------------------------
# Device kernel builder
# --------------------------------------------------------------------------
def build_kernel(nc, tc, cfg: Config, meta):
    from contextlib import ExitStack

    import concourse.mybir as mybir

    f32 = mybir.dt.float32
    bf16 = mybir.dt.bfloat16
    i32 = mybir.dt.int32
    i16 = mybir.dt.int16
    Alu = mybir.AluOpType
    Act = mybir.ActivationFunctionType

    tg, tcore, npad = cfg.t_global, cfg.t_core, cfg.n_pad
    layouts = meta["layouts"]
    cstar = meta["cstar"]
    use_bias = meta["use_bias"]
    n_chunks = [layouts[g][3] for g in range(2)]
    ncols = [n_chunks[g] * P // 16 for g in range(2)]

    featst = nc.dram_tensor("featst", [P, npad], f32, kind="ExternalInput").ap()
    w_in = nc.dram_tensor("w_in", [P, D], f32, kind="ExternalInput").ap()
    a_rep = nc.dram_tensor("a_rep", [P, 1], f32, kind="ExternalInput").ap()
    b_rep = nc.dram_tensor("b_rep", [P, D], f32, kind="ExternalInput").ap()
    dego = nc.dram_tensor("dego", [P, 2 * tg], i32, kind="ExternalInput").ap()
    degi = nc.dram_tensor("degi", [P, 2 * tcore], i32, kind="ExternalInput").ap()
    idx_in = nc.dram_tensor("idx_in", [P, sum(ncols)], i16,
                            kind="ExternalInput").ap()
    off_in = nc.dram_tensor("off_in", [P, sum(n_chunks)], f32,
                            kind="ExternalInput").ap()
    iota_in = nc.dram_tensor("iota_in", [P, P], bf16, kind="ExternalInput").ap()
    out = nc.dram_tensor("out", [2, tcore, P, D], f32, kind="ExternalOutput").ap()

    y_dram = [nc.dram_tensor(f"y{g}", [npad, D], bf16, kind="Internal").ap()
              for g in range(2)]

    with ExitStack() as ctx:
        const = ctx.enter_context(tc.tile_pool(name="const", bufs=1))
        work = ctx.enter_context(tc.tile_pool(name="work", bufs=2))
        xpool = ctx.enter_context(tc.tile_pool(name="xpool", bufs=3))
        mpool = ctx.enter_context(tc.tile_pool(name="mpool", bufs=3))
        gpool = ctx.enter_context(tc.tile_pool(name="gpool", bufs=cfg.gbufs))
        import os as _os
        ipool = ctx.enter_context(tc.tile_pool(
            name="ipool", bufs=int(_os.environ.get("IPB", "3"))))
        ohpool = ctx.enter_context(tc.tile_pool(name="ohpool", bufs=6))
        tpool = ctx.enter_context(tc.tile_pool(name="tpool", bufs=4))
        spool = ctx.enter_context(tc.tile_pool(name="spool", bufs=3))
        ypool = ctx.enter_context(tc.tile_pool(
            name="ypool", bufs=int(_os.environ.get("YPB", "4")), space="PSUM"))
        ppool = ctx.enter_context(tc.tile_pool(
            name="ppool", bufs=int(_os.environ.get("PPB", "4")), space="PSUM"))

        # ---- constants ----
        w_sb = const.tile([P, D], bf16)
        nc.gpsimd.dma_start(out=w_sb[:], in_=w_in)  # f32 -> bf16 cast DMA
        iota_sb = const.tile([P, P], bf16)
        nc.sync.dma_start(out=iota_sb[:], in_=iota_in)
        a_sb = const.tile([P, 1], f32)
        nc.sync.dma_start(out=a_sb[:], in_=a_rep)
        if use_bias:
            b_sb = const.tile([P, D], f32)
            nc.sync.dma_start(out=b_sb[:], in_=b_rep)

        # ---- norms from degrees:  norm = (deg>0) / sqrt(max(deg,1)) ----
        def make_norm(deg_ap, width, tagn):
            dg = work.tile([P, width], i32, tag=f"dg{tagn}")
            nc.sync.dma_start(out=dg[:], in_=deg_ap)
            f = work.tile([P, width], f32, tag=f"f{tagn}")
            nc.vector.tensor_copy(out=f[:], in_=dg[:])
            m = work.tile([P, width], f32, tag=f"m{tagn}")
            nc.vector.tensor_scalar(out=m[:], in0=f[:], scalar1=1.0,
                                    scalar2=None, op0=Alu.max)
            r = work.tile([P, width], f32, tag=f"r{tagn}")
            nc.vector.reciprocal(out=r[:], in_=m[:])
            s = work.tile([P, width], f32, tag=f"s{tagn}")
            nc.scalar.activation(out=s[:], in_=r[:], func=Act.Sqrt)
            z = work.tile([P, width], f32, tag=f"z{tagn}")
            nc.vector.tensor_scalar(out=z[:], in0=f[:], scalar1=1.0,
                                    scalar2=None, op0=Alu.min)
            ns = const.tile([P, width], f32, tag=f"o{tagn}")
            nc.vector.tensor_tensor(out=ns[:], in0=s[:], in1=z[:], op=Alu.mult)
            return ns

        ns_sb = make_norm(dego, 2 * tg, "o")       # out-deg norms, all nodes
        nd_sb = make_norm(degi, 2 * tcore, "i")    # in-deg norms, owned slots
        and_sb = const.tile([P, 2 * tcore], f32)
        nc.vector.tensor_tensor(out=and_sb[:], in0=nd_sb[:],
                                in1=a_sb[:, :1].to_broadcast([P, 2 * tcore]),
                                op=Alu.mult)

        # ---- y-phase: y_g = bf16(ns_g * (feats @ W)) ----
        t0 = 0
        while t0 < tg:
            nb = min(cfg.xbatch, tg - t0)
            ld = xpool.tile([P, nb * P], f32, tag="xload")
            nc.sync.dma_start(out=ld[:], in_=featst[:, t0 * P : (t0 + nb) * P])
            ldb = xpool.tile([P, nb * P], bf16, tag="xcast")
            nc.vector.tensor_copy(out=ldb[:], in_=ld[:])
            ybuf0 = xpool.tile([P, nb, D], bf16, tag="ybuf0")
            ybuf1 = xpool.tile([P, nb, D], bf16, tag="ybuf1")
            ybuf = [ybuf0, ybuf1]
            for i in range(nb):
                psy = ypool.tile([P, D], f32)
                nc.tensor.matmul(out=psy[:], lhsT=ldb[:, i * P : (i + 1) * P],
                                 rhs=w_sb[:], start=True, stop=True)
                col = t0 + i
                if cfg.y_act_split:
                    nc.scalar.activation(out=ybuf[0][:, i, :], in_=psy[:],
                                         func=Act.Copy,
                                         scale=ns_sb[:, col : col + 1])
                else:
                    nc.vector.tensor_scalar(out=ybuf[0][:, i, :], in0=psy[:],
                                            scalar1=ns_sb[:, col : col + 1],
                                            scalar2=None, op0=Alu.mult)
                nc.vector.tensor_scalar(out=ybuf[1][:, i, :], in0=psy[:],
                                        scalar1=ns_sb[:, tg + col : tg + col + 1],
                                        scalar2=None, op0=Alu.mult)
            for g in range(2):
                nc.sync.dma_start(
                    out=y_dram[g][t0 * P : (t0 + nb) * P, :].rearrange(
                        "(p i) d -> p i d", i=nb),
                    in_=ybuf[g][:])
            t0 += nb

        # ---- gather + one-hot segment-sum + nd-scale + prelu ----
        col_base = [0, ncols[0]]          # idx column offset per graph
        chk_base = [0, n_chunks[0]]       # off column offset per graph
        cbs_all = []
        for g in range(2):
            calls_by_sg = {}
            for (si, b, c0, nch) in layouts[g][1]:
                calls_by_sg.setdefault(si, []).append((b, c0, nch))
            cbs_all.append(calls_by_sg)
        # interleave the two graphs' supergroups so one graph's gathers fill
        # DMA while the other's PSUM chain drains
        jobs = []
        for si in range(max(len(layouts[0][0]), len(layouts[1][0]))):
            for g in range(2):
                if si < len(layouts[g][0]):
                    jobs.append((g, si))
        for (g, si) in jobs:
            sg_list, calls, chunk_of, _ = layouts[g]
            cs = cstar[g]
            calls_by_sg = cbs_all[g]
            if True:
                (k0, kn) = sg_list[si]
                sg_chunks = sum(int(cs[k0 + ki, b]) for ki in range(kn)
                                for b in range(NBANK))
                c0_sg = chunk_of[(k0, 0, 0)]
                gt = gpool.tile([P, sg_chunks, D], bf16, tag="gather")
                it = ipool.tile([P, sg_chunks * 8], i16, tag="gidx")
                nc.sync.dma_start(
                    out=it[:],
                    in_=idx_in[:, col_base[g] + c0_sg * 8 :
                               col_base[g] + (c0_sg + sg_chunks) * 8])
                ot = ipool.tile([P, sg_chunks], f32, tag="goff")
                nc.sync.dma_start(
                    out=ot[:],
                    in_=off_in[:, chk_base[g] + c0_sg :
                               chk_base[g] + c0_sg + sg_chunks])
                for (b, c0, nch) in calls_by_sg[si]:
                    lo = c0 - c0_sg
                    bank_rows = min(cfg.bank_rows, npad - b * cfg.bank_rows)
                    nc.gpsimd.dma_gather(
                        out_ap=gt[:, lo : lo + nch, :],
                        in_ap=y_dram[g][b * cfg.bank_rows :
                                        b * cfg.bank_rows + bank_rows, :],
                        idxs_ap=it[:, lo * 8 : (lo + nch) * 8],
                        num_idxs=nch * P, num_idxs_reg=nch * P,
                        elem_size=D, single_packet=False)
                stg = spool.tile([P, kn, D], f32, tag="stg")
                for ki in range(kn):
                    k = k0 + ki
                    nonzero = [(b, c) for b in range(NBANK)
                               for c in range(int(cs[k, b]))]
                    ps_a = ppool.tile([P, D], f32)
                    for j, (b, c) in enumerate(nonzero):
                        ci = chunk_of[(k, b, c)]
                        lo = ci - c0_sg
                        oh = ohpool.tile([P, P], bf16)
                        eng = nc.vector
                        if cfg.oh_gpsimd_mod and (ci % cfg.oh_gpsimd_mod == 0):
                            eng = nc.gpsimd
                        eng.tensor_scalar(
                            out=oh[:], in0=iota_sb[:],
                            scalar1=ot[:, lo : lo + 1],
                            scalar2=None, op0=Alu.is_equal)
                        nc.tensor.matmul(
                            out=ps_a[:], lhsT=oh[:], rhs=gt[:, lo, :],
                            start=(j == 0), stop=(j == len(nonzero) - 1))
                    kslot = g * tcore + k
                    if cfg.act_prelu and not use_bias:
                        nc.scalar.activation(
                            out=stg[:, ki, :], in_=ps_a[:], func=Act.Prelu,
                            scale=nd_sb[:, kslot : kslot + 1],
                            alpha=a_sb[:, :1])
                        continue
                    if use_bias:
                        hb = tpool.tile([P, D], f32, tag="hb")
                        nc.vector.tensor_scalar(
                            out=hb[:], in0=ps_a[:],
                            scalar1=nd_sb[:, kslot : kslot + 1],
                            scalar2=None, op0=Alu.mult)
                        hb2 = tpool.tile([P, D], f32, tag="hb2")
                        nc.vector.tensor_tensor(out=hb2[:], in0=hb[:],
                                                in1=b_sb[:], op=Alu.add)
                        neg = tpool.tile([P, D], f32, tag="neg")
                        nc.vector.tensor_scalar(
                            out=neg[:], in0=hb2[:], scalar1=0.0,
                            scalar2=a_sb[:, :1], op0=Alu.min, op1=Alu.mult)
                        pos = tpool.tile([P, D], f32, tag="pos")
                        nc.vector.tensor_scalar(
                            out=pos[:], in0=hb2[:], scalar1=0.0,
                            scalar2=None, op0=Alu.max)
                    else:
                        neg = tpool.tile([P, D], f32, tag="neg")
                        nc.vector.tensor_scalar(
                            out=neg[:], in0=ps_a[:], scalar1=0.0,
                            scalar2=and_sb[:, kslot : kslot + 1],
                            op0=Alu.min, op1=Alu.mult)
                        pos = tpool.tile([P, D], f32, tag="pos")
                        nc.vector.tensor_scalar(
                            out=pos[:], in0=ps_a[:], scalar1=0.0,
                            scalar2=nd_sb[:, kslot : kslot + 1],
                            op0=Alu.max, op1=Alu.mult)
                    nc.vector.tensor_tensor(out=stg[:, ki, :], in0=neg[:],
                                            in1=pos[:], op=Alu.add)
                nc.sync.dma_start(
                    out=out[g, k0 : k0 + kn, :, :].rearrange("k p d -> p k d"),
                    in_=stg[:])
    return out


# --------------------------------------------------------------------------
# Driver
# --------------------------------------------------------------------------
def _build_program(cfg: Config, meta):
    import concourse.bacc as bacc
    import concourse.tile as tile

    nc = bacc.Bacc("TRN2", target_bir_lowering=False, debug=False,
                   enable_asserts=False, num_devices=cfg.n_cores)
    with tile.TileContext(nc) as tc:
        build_kernel(nc, tc, cfg, meta)
    nc.compile()
    return nc


def _unscramble(results, plans, cfg: Config):
    n = cfg.n_nodes
    full = np.zeros((2, n, D), np.float32)
    for g in range(2):
        ct_all = plans[g]["core_tiles"]
        for core in range(cfg.n_cores):
            oc = results[core]["out"]  # [2, t_core, P, D]
            for k in range(cfg.t_core):
                t = int(ct_all[core, k])
                if t < 0:
                    continue
                r0 = t * P
                r1 = min(r0 + P, n)
                full[g, r0:r1] = oc[g, k, : r1 - r0, :]
    return full


_PROGRAM_CACHE = {}


def run(inputs, cfg: Config, trace=False):
    from concourse.bass_utils import run_bass_kernel_spmd

    in_maps, plans, meta = preprocess(
        inputs["feats"], inputs["W"], inputs["b"], inputs["prelu_a"],
        inputs["src_pos"], inputs["dst_pos"],
        inputs["src_neg"], inputs["dst_neg"], cfg)

    key = (cfg.n_nodes, cfg.n_cores, cfg.xbatch, cfg.sg, cfg.y_act_split,
           cfg.act_prelu, cfg.oh_gpsimd_mod, cfg.gbufs,
           meta["cstar"][0].tobytes(), meta["cstar"][1].tobytes(),
           meta["use_bias"])
    nc = _PROGRAM_CACHE.get(key)
    if nc is None:
        nc = _build_program(cfg, meta)
        _PROGRAM_CACHE[key] = nc

    kwargs = {}
    if trace:
        kwargs = dict(trace=True, tmpdir=tempfile.mkdtemp(prefix="bgc_trace_"))
    res = run_bass_kernel_spmd(nc, in_maps, core_ids=list(range(cfg.n_cores)),
                               **kwargs)
    full = _unscramble(res.results, plans, cfg)
    return full, res


def kernel(**inputs) -> np.ndarray:
    cfg = Config()
    full, _ = run(inputs, cfg)
    return full



# revision 2
# speedup vs baseline: 1.2756x; 1.2756x over previous
"""Bass/Trainium2 kernel for BiGraphContrastLayer (GNN message passing).

Computes, for two edge lists (pos/neg) over the same node features:
    h_g = PReLU( D_in^-1/2 A_g D_out^-1/2 feats @ W + b )
returning stack([h_pos, h_neg]) of shape [2, N, Dout].

Strategy (8 NeuronCores, SPMD, no collectives), using the linearity
   (D_in^-1/2 A D_out^-1/2 feats) @ W = D_in^-1/2 A (D_out^-1/2 feats W):

  Phase 1 (y-phase): y_raw = bf16(feats @ W), UNSCALED and shared by both
    graphs. feats comes in host-transposed and pre-cast to bf16 (featsT is
    directly the matmul lhsT); 4 matmuls accumulate into one PSUM bank,
    one engine copy casts [128,512] PSUM->SBUF bf16, DMA to DRAM in a
    p-major row layout (2KB contiguous per partition).
  Phase 2 (gather/aggregate): edges are bucketed by 128-node dst tile and
    sorted by src. dma_gather (int16, 4 row-banks of <=25088 rows) pulls
    y_raw[src] rows for ~128-edge chunks; a weighted one-hot matmul
    (lhsT[p, dstoff] = ns[src_p]*nd[dst_p], built on DVE via
    is_equal+mult) segment-sums AND norm-scales each chunk into the dst
    tile's PSUM accumulator. Final PReLU on ScalarE, bf16 store.

  Host does index/metadata work only: degree bincounts -> per-edge norm
  weights, sorting, bucketing, dealing dst tiles to cores so all 8 cores
  share one instruction stream (signature-matched by per-bank chunk
  counts), building int16 wrapped gather indices, and replicating the
  small W/b/prelu params per the sharding hint.
"""

import math
import tempfile
from dataclasses import dataclass

import numpy as np

P = 128   # partitions
D = 128   # feature dim (Din == Dout == 128)
NBANK = 4
BF16 = np.dtype("bfloat16")


# --------------------------------------------------------------------------
# Config
# --------------------------------------------------------------------------
@dataclass
class Config:
    n_nodes: int = 100000
    n_cores: int = 8
    xbatch: int = 8    # feats tiles per y-phase batch (must be mult of 4)
    sg: int = 10       # dst-tile positions per gather supergroup
    act_prelu: bool = True    # final PReLU on ScalarE (not in sim)
    oh_gpsimd_mod: int = 0    # every Nth one-hot build goes to GpSimd (0=off)
    gbufs: int = 2            # gather buffer count
    ipbufs: int = 3           # idx buffer count
    ypbufs: int = 4           # y-phase PSUM bank count
    ppbufs: int = 4           # gather-phase PSUM bank count

    @property
    def t_global(self) -> int:
        return math.ceil(self.n_nodes / P)

    @property
    def n_pad(self) -> int:
        return self.t_global * P

    @property
    def t_core(self) -> int:
        return math.ceil(self.t_global / self.n_cores)

    @property
    def bank_tiles(self) -> int:
        return math.ceil(self.t_global / NBANK)

    @property
    def bank_rows(self) -> int:
        return self.bank_tiles * P


# --------------------------------------------------------------------------
# Host-side preprocessing (integer index / edge-weight metadata only)
# --------------------------------------------------------------------------
def _row_of(n, cfg: Config):
    """y DRAM row of node n: within each xbatch of tiles, rows are laid
    p-major (node (t0+i)*128+p -> row t0*128 + p*nb + i) so the y-phase
    store writes nb*256B contiguous per partition."""
    xb, tg = cfg.xbatch, cfg.t_global
    t = n // P
    p = n % P
    t0 = (t // xb) * xb
    nb = np.minimum(xb, tg - t0)
    return t0 * P + p * nb + (t - t0)


def _plan_graph(src, dst, nse_edge, cfg: Config):
    """Bucket edges by dst tile, sort by src within tile, bank-split, and
    deal tiles to cores with per-bank-signature matching.

    Returns dict with:
      core_tiles  [n_cores, t_core]  global tile id per position (-1 null)
      cstar       [t_core, NBANK]    shared chunk counts per position/bank
      tile_edges  list per global tile: (src_sorted, off_sorted, nse_sorted,
                                         bank_cnt)
    """
    tg, ncores, tcore = cfg.t_global, cfg.n_cores, cfg.t_core
    order = np.argsort(dst, kind="stable")
    src_s = src[order]
    dst_s = dst[order]
    nse_s = nse_edge[order]
    tile_cnt = np.bincount(dst_s // P, minlength=tg)
    starts = np.zeros(tg + 1, np.int64)
    np.cumsum(tile_cnt, out=starts[1:])

    row_s = _row_of(src_s, cfg)
    bank_of = row_s // cfg.bank_rows
    sig = np.zeros((tg, NBANK), np.int64)
    tile_edges = []
    for t in range(tg):
        e0, e1 = int(starts[t]), int(starts[t + 1])
        so = np.argsort(row_s[e0:e1], kind="stable")
        ts_src = row_s[e0:e1][so]
        ts_off = (dst_s[e0:e1][so] % P).astype(np.int64)
        ts_nse = nse_s[e0:e1][so]
        bc = np.bincount(bank_of[e0:e1], minlength=NBANK)
        sig[t] = -(-bc // P)  # ceil chunks per bank
        sig[t, 0] = max(sig[t, 0], 1)
        tile_edges.append((ts_src, ts_off, ts_nse, bc))

    # Deal: sort tiles by signature so consecutive groups of n_cores tiles
    # have matching/near-matching per-bank chunk counts.
    sigkey = sig @ (np.int64(32) ** np.arange(NBANK - 1, -1, -1))
    keys = np.argsort(sigkey, kind="stable")
    # pad with null tiles to n_cores * t_core
    n_slots = ncores * tcore
    dealt = np.full(n_slots, -1, np.int64)
    dealt[: len(keys)] = keys[::-1]  # descending signature order
    core_tiles = np.zeros((ncores, tcore), np.int64)
    cstar = np.zeros((tcore, NBANK), np.int64)
    for k in range(tcore):
        grp = dealt[k * ncores : (k + 1) * ncores]
        for c in range(ncores):
            core_tiles[c, k] = grp[c]
        s = np.zeros(NBANK, np.int64)
        for t in grp:
            if t >= 0:
                s = np.maximum(s, sig[t])
        s[0] = max(s[0], 1)
        cstar[k] = s
    return dict(core_tiles=core_tiles, cstar=cstar, tile_edges=tile_edges)


def _slot_layout(cstar, cfg: Config):
    """Shared (all-cores) slot layout for one graph.

    Slot space = sequence of supergroups; within a supergroup, bank-major:
      for b in banks: for k in sg positions: cstar[k, b] chunks.
    Returns:
      sg_list: list of (k0, kn)
      calls:   list of (sg_idx, bank, chunk0, nchunks)  [nchunks > 0]
      chunk_of: dict (k, b, c) -> global chunk index
      n_chunks total
    """
    tcore, sg = cfg.t_core, cfg.sg
    sg_list = []
    k0 = 0
    while k0 < tcore:
        kn = min(sg, tcore - k0)
        sg_list.append((k0, kn))
        k0 += kn
    calls = []
    chunk_of = {}
    cpos = 0
    for si, (k0, kn) in enumerate(sg_list):
        for b in range(NBANK):
            c0 = cpos
            for ki in range(kn):
                k = k0 + ki
                for c in range(int(cstar[k, b])):
                    chunk_of[(k, b, c)] = cpos
                    cpos += 1
            if cpos > c0:
                calls.append((si, b, c0, cpos - c0))
    return sg_list, calls, chunk_of, cpos


def _fill_core_graph(plan, layout, core, cfg: Config):
    """Build IDX16 (wrapped), OFF and NSE arrays for one core, one graph."""
    sg_list, calls, chunk_of, n_chunks = layout
    cstar = plan["cstar"]
    idx = np.zeros((n_chunks, P), np.int16)
    off = np.full((n_chunks, P), 512.0, np.float32)
    nse = np.zeros((n_chunks, P), np.float32)
    for k in range(cfg.t_core):
        t = plan["core_tiles"][core, k]
        if t < 0:
            continue
        ts_src, ts_off, ts_nse, bc = plan["tile_edges"][t]
        bstart = np.zeros(NBANK + 1, np.int64)
        np.cumsum(bc, out=bstart[1:])
        for b in range(NBANK):
            nb = int(bc[b])
            cnum = int(cstar[k, b])
            if cnum == 0:
                continue
            nslot = cnum * P
            es = np.zeros(nslot, np.int64)
            eo = np.full(nslot, 512.0, np.float32)
            en = np.zeros(nslot, np.float32)
            es[:nb] = ts_src[bstart[b] : bstart[b + 1]] - b * cfg.bank_rows
            eo[:nb] = ts_off[bstart[b] : bstart[b + 1]]
            en[:nb] = ts_nse[bstart[b] : bstart[b + 1]]
            for c in range(cnum):
                ci = chunk_of[(k, b, c)]
                idx[ci] = es[c * P : (c + 1) * P]
                off[ci] = eo[c * P : (c + 1) * P]
                nse[ci] = en[c * P : (c + 1) * P]
    # wrap: flat slot i (within a call's range) -> [i%16, i//16], replicated
    # to 128 partitions. Build per call, concatenated along columns.
    ncols = n_chunks * P // 16
    idx_w = np.zeros((P, ncols), np.int16)
    for (si, b, c0, nch) in calls:
        flat = idx[c0 : c0 + nch].reshape(-1)  # ni slots
        blk = flat.reshape(-1, 16).T  # [16, ni/16]
        idx_w[:, c0 * 8 : (c0 + nch) * 8] = np.tile(blk, (8, 1))
    return idx_w, off.T.copy(), nse.T.copy()  # -> [P, n_chunks] f32


def preprocess(feats, W, b, prelu_a, src_pos, dst_pos, src_neg, dst_neg,
               cfg: Config):
    n, ncores = cfg.n_nodes, cfg.n_cores
    feats = np.asarray(feats, np.float32)
    W = np.asarray(W, np.float32)
    b = np.asarray(b, np.float32)
    prelu_a = np.asarray(prelu_a, np.float32)

    feats_pad = np.zeros((cfg.n_pad, D), np.float32)
    feats_pad[:n] = feats
    featsT = np.ascontiguousarray(feats_pad.T).astype(BF16)  # [D, n_pad] bf16

    plans, layouts = [], []
    for src, dst in ((src_pos, dst_pos), (src_neg, dst_neg)):
        src = np.asarray(src, np.int64)
        dst = np.asarray(dst, np.int64)
        deg_out = np.bincount(src, minlength=n).astype(np.float32)
        deg_in = np.bincount(dst, minlength=n).astype(np.float32)
        ns = np.where(deg_out > 0, 1.0 / np.sqrt(np.maximum(deg_out, 1.0)),
                      0.0).astype(np.float32)
        nd = np.where(deg_in > 0, 1.0 / np.sqrt(np.maximum(deg_in, 1.0)),
                      0.0).astype(np.float32)
        nse_edge = ns[src] * nd[dst]
        plan = _plan_graph(src, dst, nse_edge, cfg)
        plans.append(plan)
        layouts.append(_slot_layout(plan["cstar"], cfg))

    iota = np.tile(np.arange(P, dtype=np.float32), (P, 1)).astype(BF16)
    a_rep = np.full((P, 1), float(prelu_a.reshape(-1)[0]), np.float32)
    b_rep = np.tile(b.reshape(1, D), (P, 1)).astype(np.float32)

    in_maps = []
    for core in range(ncores):
        iw_p, off_p, nse_p = _fill_core_graph(plans[0], layouts[0], core, cfg)
        iw_n, off_n, nse_n = _fill_core_graph(plans[1], layouts[1], core, cfg)
        in_maps.append({
            "featst": featsT,
            "w_in": W,
            "a_rep": a_rep,
            "b_rep": b_rep,
            "idx_in": np.concatenate([iw_p, iw_n], axis=1),
            "off_in": np.concatenate([off_p, off_n], axis=1),
            "nse_in": np.concatenate([nse_p, nse_n], axis=1),
            "iota_in": iota,
        })
    meta = {
        "layouts": layouts,
        "cstar": [plans[0]["cstar"], plans[1]["cstar"]],
        "use_bias": bool(np.any(b != 0.0)),
    }
    return in_maps, plans, meta


# --------------------------------------------------------------------------
# Device kernel builder
# --------------------------------------------------------------------------
def build_kernel(nc, tc, cfg: Config, meta):
    from contextlib import ExitStack

    import concourse.mybir as mybir

    f32 = mybir.dt.float32
    bf16 = mybir.dt.bfloat16
    i16 = mybir.dt.int16
    Alu = mybir.AluOpType
    Act = mybir.ActivationFunctionType

    tg, tcore, npad = cfg.t_global, cfg.t_core, cfg.n_pad
    layouts = meta["layouts"]
    cstar = meta["cstar"]
    use_bias = meta["use_bias"]
    n_chunks = [layouts[g][3] for g in range(2)]
    ncols = [n_chunks[g] * P // 16 for g in range(2)]

    featst = nc.dram_tensor("featst", [P, npad], bf16, kind="ExternalInput").ap()
    w_in = nc.dram_tensor("w_in", [P, D], f32, kind="ExternalInput").ap()
    a_rep = nc.dram_tensor("a_rep", [P, 1], f32, kind="ExternalInput").ap()
    b_rep = nc.dram_tensor("b_rep", [P, D], f32, kind="ExternalInput").ap()
    idx_in = nc.dram_tensor("idx_in", [P, sum(ncols)], i16,
                            kind="ExternalInput").ap()
    off_in = nc.dram_tensor("off_in", [P, sum(n_chunks)], f32,
                            kind="ExternalInput").ap()
    nse_in = nc.dram_tensor("nse_in", [P, sum(n_chunks)], f32,
                            kind="ExternalInput").ap()
    iota_in = nc.dram_tensor("iota_in", [P, P], bf16, kind="ExternalInput").ap()
    out = nc.dram_tensor("out", [2, tcore, P, D], bf16,
                         kind="ExternalOutput").ap()

    y_dram = nc.dram_tensor("y", [npad, D], bf16, kind="Internal").ap()

    with ExitStack() as ctx:
        const = ctx.enter_context(tc.tile_pool(name="const", bufs=1))
        xpool = ctx.enter_context(tc.tile_pool(name="xpool", bufs=3))
        gpool = ctx.enter_context(tc.tile_pool(name="gpool", bufs=cfg.gbufs))
        ipool = ctx.enter_context(tc.tile_pool(name="ipool", bufs=cfg.ipbufs))
        ohpool = ctx.enter_context(tc.tile_pool(name="ohpool", bufs=6))
        tpool = ctx.enter_context(tc.tile_pool(name="tpool", bufs=4))
        spool = ctx.enter_context(tc.tile_pool(name="spool", bufs=3))
        ypool = ctx.enter_context(tc.tile_pool(
            name="ypool", bufs=cfg.ypbufs, space="PSUM"))
        ppool = ctx.enter_context(tc.tile_pool(
            name="ppool", bufs=cfg.ppbufs, space="PSUM"))

        # ---- constants ----
        w_sb = const.tile([P, D], bf16)
        nc.gpsimd.dma_start(out=w_sb[:], in_=w_in)  # f32 -> bf16 cast DMA
        iota_sb = const.tile([P, P], bf16)
        nc.sync.dma_start(out=iota_sb[:], in_=iota_in)
        a_sb = const.tile([P, 1], f32)
        nc.sync.dma_start(out=a_sb[:], in_=a_rep)
        if use_bias:
            b_sb = const.tile([P, D], f32)
            nc.sync.dma_start(out=b_sb[:], in_=b_rep)

        # ---- y-phase: y_raw = bf16(feats @ W), unscaled, shared ----
        assert cfg.xbatch % 4 == 0
        t0 = 0
        yphase = 0
        while t0 < tg:
            nb = min(cfg.xbatch, tg - t0)
            ld = xpool.tile([P, nb * P], bf16, tag="xload")
            nc.sync.dma_start(out=ld[:], in_=featst[:, t0 * P : (t0 + nb) * P])
            ybuf = xpool.tile([P, nb, D], bf16, tag="ybuf")
            i = 0
            while i < nb:
                qn = min(4, nb - i)
                psy = ypool.tile([P, qn * D], f32)
                for q in range(qn):
                    nc.tensor.matmul(
                        out=psy[:, q * D : (q + 1) * D],
                        lhsT=ld[:, (i + q) * P : (i + q + 1) * P],
                        rhs=w_sb[:], start=True, stop=True)
                # cast PSUM f32 -> SBUF bf16, alternate engines
                dst = ybuf[:, i : i + qn, :].rearrange("p i d -> p (i d)")
                if yphase % 2 == 0:
                    nc.scalar.activation(out=dst, in_=psy[:], func=Act.Copy)
                else:
                    nc.vector.tensor_copy(out=dst, in_=psy[:])
                yphase += 1
                i += qn
            nc.sync.dma_start(
                out=y_dram[t0 * P : (t0 + nb) * P, :].rearrange(
                    "(p i) d -> p i d", i=nb),
                in_=ybuf[:])
            t0 += nb

        # ---- gather + weighted one-hot segment-sum + prelu ----
        col_base = [0, ncols[0]]          # idx column offset per graph
        chk_base = [0, n_chunks[0]]       # off/nse column offset per graph
        cbs_all = []
        for g in range(2):
            calls_by_sg = {}
            for (si, b, c0, nch) in layouts[g][1]:
                calls_by_sg.setdefault(si, []).append((b, c0, nch))
            cbs_all.append(calls_by_sg)
        # interleave the two graphs' supergroups so one graph's gathers fill
        # DMA while the other's PSUM chain drains
        jobs = []
        for si in range(max(len(layouts[0][0]), len(layouts[1][0]))):
            for g in range(2):
                if si < len(layouts[g][0]):
                    jobs.append((g, si))
        for (g, si) in jobs:
            sg_list, calls, chunk_of, _ = layouts[g]
            cs = cstar[g]
            calls_by_sg = cbs_all[g]
            (k0, kn) = sg_list[si]
            sg_chunks = sum(int(cs[k0 + ki, b]) for ki in range(kn)
                            for b in range(NBANK))
            c0_sg = chunk_of[(k0, 0, 0)]
            gt = gpool.tile([P, sg_chunks, D], bf16, tag="gather")
            it = ipool.tile([P, sg_chunks * 8], i16, tag="gidx")
            nc.sync.dma_start(
                out=it[:],
                in_=idx_in[:, col_base[g] + c0_sg * 8 :
                           col_base[g] + (c0_sg + sg_chunks) * 8])
            ot = ipool.tile([P, sg_chunks], f32, tag="goff")
            nc.sync.dma_start(
                out=ot[:],
                in_=off_in[:, chk_base[g] + c0_sg :
                           chk_base[g] + c0_sg + sg_chunks])
            et = ipool.tile([P, sg_chunks], f32, tag="gnse")
            nc.sync.dma_start(
                out=et[:],
                in_=nse_in[:, chk_base[g] + c0_sg :
                           chk_base[g] + c0_sg + sg_chunks])
            for (b, c0, nch) in calls_by_sg[si]:
                lo = c0 - c0_sg
                bank_rows = min(cfg.bank_rows, npad - b * cfg.bank_rows)
                nc.gpsimd.dma_gather(
                    out_ap=gt[:, lo : lo + nch, :],
                    in_ap=y_dram[b * cfg.bank_rows :
                                 b * cfg.bank_rows + bank_rows, :],
                    idxs_ap=it[:, lo * 8 : (lo + nch) * 8],
                    num_idxs=nch * P, num_idxs_reg=nch * P,
                    elem_size=D, single_packet=False)
            stg = spool.tile([P, kn, D], bf16, tag="stg")
            for ki in range(kn):
                k = k0 + ki
                nonzero = [(b, c) for b in range(NBANK)
                           for c in range(int(cs[k, b]))]
                ps_a = ppool.tile([P, D], f32)
                for j, (b, c) in enumerate(nonzero):
                    ci = chunk_of[(k, b, c)]
                    lo = ci - c0_sg
                    oh = ohpool.tile([P, P], bf16)
                    eng = nc.vector
                    if cfg.oh_gpsimd_mod and (ci % cfg.oh_gpsimd_mod == 0):
                        eng = nc.gpsimd
                    eng.tensor_scalar(
                        out=oh[:], in0=iota_sb[:],
                        scalar1=ot[:, lo : lo + 1],
                        scalar2=et[:, lo : lo + 1],
                        op0=Alu.is_equal, op1=Alu.mult)
                    nc.tensor.matmul(
                        out=ps_a[:], lhsT=oh[:], rhs=gt[:, lo, :],
                        start=(j == 0), stop=(j == len(nonzero) - 1))
                if use_bias:
                    hb2 = tpool.tile([P, D], f32, tag="hb2")
                    nc.vector.tensor_tensor(out=hb2[:], in0=ps_a[:],
                                            in1=b_sb[:], op=Alu.add)
                    neg = tpool.tile([P, D], f32, tag="neg")
                    nc.vector.tensor_scalar(
                        out=neg[:], in0=hb2[:], scalar1=0.0,
                        scalar2=a_sb[:, :1], op0=Alu.min, op1=Alu.mult)
                    pos = tpool.tile([P, D], f32, tag="pos")
                    nc.vector.tensor_scalar(
                        out=pos[:], in0=hb2[:], scalar1=0.0,
                        scalar2=None, op0=Alu.max)
                    nc.vector.tensor_tensor(out=stg[:, ki, :], in0=neg[:],
                                            in1=pos[:], op=Alu.add)
                elif cfg.act_prelu:
                    nc.scalar.activation(
                        out=stg[:, ki, :], in_=ps_a[:], func=Act.Prelu,
                        alpha=a_sb[:, :1])
                else:
                    neg = tpool.tile([P, D], f32, tag="neg")
                    nc.vector.tensor_scalar(
                        out=neg[:], in0=ps_a[:], scalar1=0.0,
                        scalar2=a_sb[:, :1], op0=Alu.min, op1=Alu.mult)
                    pos = tpool.tile([P, D], f32, tag="pos")
                    nc.vector.tensor_scalar(
                        out=pos[:], in0=ps_a[:], scalar1=0.0,
                        scalar2=None, op0=Alu.max)
                    nc.vector.tensor_tensor(out=stg[:, ki, :], in0=neg[:],
                                            in1=pos[:], op=Alu.add)
            nc.sync.dma_start(
                out=out[g, k0 : k0 + kn, :, :].rearrange("k p d -> p k d"),
                in_=stg[:])
    return out


# --------------------------------------------------------------------------
# Driver
# --------------------------------------------------------------------------
def _build_program(cfg: Config, meta):
    import concourse.bacc as bacc
    import concourse.tile as tile

    nc = bacc.Bacc("TRN2", target_bir_lowering=False, debug=False,
                   enable_asserts=False, num_devices=cfg.n_cores)
    with tile.TileContext(nc) as tc:
        build_kernel(nc, tc, cfg, meta)
    nc.compile()
    return nc


def _unscramble(results, plans, cfg: Config):
    n = cfg.n_nodes
    full = np.zeros((2, n, D), np.float32)
    for g in range(2):
        ct_all = plans[g]["core_tiles"]
        for core in range(cfg.n_cores):
            oc = np.asarray(results[core]["out"], dtype=np.float32)
            for k in range(cfg.t_core):
                t = int(ct_all[core, k])
                if t < 0:
                    continue
                r0 = t * P
                r1 = min(r0 + P, n)
                full[g, r0:r1] = oc[g, k, : r1 - r0, :]
    return full


_PROGRAM_CACHE = {}


def run(inputs, cfg: Config, trace=False):
    from concourse.bass_utils import run_bass_kernel_spmd

    in_maps, plans, meta = preprocess(
        inputs["feats"], inputs["W"], inputs["b"], inputs["prelu_a"],
        inputs["src_pos"], inputs["dst_pos"],
        inputs["src_neg"], inputs["dst_neg"], cfg)

    key = (cfg.n_nodes, cfg.n_cores, cfg.xbatch, cfg.sg,
           cfg.act_prelu, cfg.oh_gpsimd_mod, cfg.gbufs,
           meta["cstar"][0].tobytes(), meta["cstar"][1].tobytes(),
           meta["use_bias"])
    nc = _PROGRAM_CACHE.get(key)
    if nc is None:
        nc = _build_program(cfg, meta)
        _PROGRAM_CACHE[key] = nc

    kwargs = {}
    if trace:
        kwargs = dict(trace=True, tmpdir=tempfile.mkdtemp(prefix="bgc_trace_"))
    res = run_bass_kernel_spmd(nc, in_maps, core_ids=list(range(cfg.n_cores)),
                               **kwargs)
    full = _unscramble(res.results, plans, cfg)
    return full, res


def kernel(**inputs) -> np.ndarray:
    cfg = Config()
    full, _ = run(inputs, cfg)
    return full


# revision 6
# speedup vs baseline: 1.7820x; 1.3970x over previous
"""Bass/Trainium2 kernel for BiGraphContrastLayer (GNN message passing).

Computes, for two edge lists (pos/neg) over the same node features:
    h_g = PReLU( D_in^-1/2 A_g D_out^-1/2 feats @ W + b )
returning stack([h_pos, h_neg]) of shape [2, N, Dout].

Strategy (8 NeuronCores, SPMD, no collectives), using the linearity
   (D_in^-1/2 A D_out^-1/2 feats) @ W = (D_in^-1/2 A D_out^-1/2 feats) W:

  No y-phase at all: dma_gather (int16 idx, 4 row-banks of <=25088 rows)
  pulls RAW bf16 feats rows straight from the (host-cast, padded) input
  for ~128-edge chunks bucketed by dst tile. A weighted one-hot matmul
  (rhs[p, dstoff] = ns[src_p]*nd[dst_p], built on DVE via is_equal+mult;
  lhsT = the gathered chunk) segment-sums AND norm-scales each chunk into
  a TRANSPOSED PSUM accumulator aggT[feat, dst]. Per dst tile: one cast
  copy aggT -> SBUF bf16, one matmul (lhsT=aggT, rhs=W) -> h PSUM, PReLU
  on ScalarE, bf16 store.

  Host does index/metadata work only: degree bincounts -> per-edge norm
  weights, sorting, bucketing, dealing dst tiles to cores so all 8 cores
  share one instruction stream (signature-matched by per-bank chunk
  counts), building int16 wrapped gather indices, and replicating the
  small W/b/prelu params per the sharding hint.
"""

import math
import tempfile
from dataclasses import dataclass

import numpy as np

P = 128   # partitions
D = 128   # feature dim (Din == Dout == 128)
NBANK = 4
BF16 = np.dtype("bfloat16")


# --------------------------------------------------------------------------
# Config
# --------------------------------------------------------------------------
@dataclass
class Config:
    n_nodes: int = 100000
    n_cores: int = 8
    sg: int = 10       # dst-tile positions per gather supergroup
    oh_gpsimd_mod: int = 0    # every Nth one-hot build goes to GpSimd (0=off)
    act_prelu: bool = True    # final PReLU on ScalarE (not in sim)
    gbufs: int = 2            # gather buffer count
    ipbufs: int = 3           # idx buffer count

    @property
    def t_global(self) -> int:
        return math.ceil(self.n_nodes / P)

    @property
    def n_pad(self) -> int:
        return self.t_global * P

    @property
    def t_core(self) -> int:
        return math.ceil(self.t_global / self.n_cores)

    @property
    def bank_tiles(self) -> int:
        return math.ceil(self.t_global / NBANK)

    @property
    def bank_rows(self) -> int:
        return self.bank_tiles * P


# --------------------------------------------------------------------------
# Host-side preprocessing (integer index / edge-weight metadata only)
# --------------------------------------------------------------------------
def _plan_graph(src, dst, nse_edge, cfg: Config):
    """Bucket edges by dst tile, sort by src within tile, bank-split, and
    deal tiles to cores with per-bank-signature matching.

    Returns dict with:
      core_tiles  [n_cores, t_core]  global tile id per position (-1 null)
      cstar       [t_core, NBANK]    shared chunk counts per position/bank
      tile_edges  list per global tile: (src_sorted, off_sorted, nse_sorted,
                                         bank_cnt)
    """
    tg, ncores, tcore = cfg.t_global, cfg.n_cores, cfg.t_core
    order = np.argsort(dst, kind="stable")
    src_s = src[order]
    dst_s = dst[order]
    nse_s = nse_edge[order]
    tile_cnt = np.bincount(dst_s // P, minlength=tg)
    starts = np.zeros(tg + 1, np.int64)
    np.cumsum(tile_cnt, out=starts[1:])

    bank_of = src_s // cfg.bank_rows
    sig = np.zeros((tg, NBANK), np.int64)
    tile_edges = []
    for t in range(tg):
        e0, e1 = int(starts[t]), int(starts[t + 1])
        so = np.argsort(src_s[e0:e1], kind="stable")
        ts_src = src_s[e0:e1][so]
        ts_off = (dst_s[e0:e1][so] % P).astype(np.int64)
        ts_nse = nse_s[e0:e1][so]
        bc = np.bincount(bank_of[e0:e1], minlength=NBANK)
        sig[t] = -(-bc // P)  # ceil chunks per bank
        sig[t, 0] = max(sig[t, 0], 1)
        tile_edges.append((ts_src, ts_off, ts_nse, bc))

    # Deal: sort tiles by signature so consecutive groups of n_cores tiles
    # have matching/near-matching per-bank chunk counts.
    sigkey = sig @ (np.int64(32) ** np.arange(NBANK - 1, -1, -1))
    keys = np.argsort(sigkey, kind="stable")
    # pad with null tiles to n_cores * t_core
    n_slots = ncores * tcore
    dealt = np.full(n_slots, -1, np.int64)
    dealt[: len(keys)] = keys[::-1]  # descending signature order
    core_tiles = np.zeros((ncores, tcore), np.int64)
    cstar = np.zeros((tcore, NBANK), np.int64)
    for k in range(tcore):
        grp = dealt[k * ncores : (k + 1) * ncores]
        for c in range(ncores):
            core_tiles[c, k] = grp[c]
        s = np.zeros(NBANK, np.int64)
        for t in grp:
            if t >= 0:
                s = np.maximum(s, sig[t])
        s[0] = max(s[0], 1)
        cstar[k] = s
    return dict(core_tiles=core_tiles, cstar=cstar, tile_edges=tile_edges)


def _slot_layout(cstar, cfg: Config):
    """Shared (all-cores) slot layout for one graph.

    Slot space = sequence of supergroups; within a supergroup, bank-major:
      for b in banks: for k in sg positions: cstar[k, b] chunks.
    Returns:
      sg_list: list of (k0, kn)
      calls:   list of (sg_idx, bank, chunk0, nchunks)  [nchunks > 0]
      chunk_of: dict (k, b, c) -> global chunk index
      n_chunks total
    """
    tcore, sg = cfg.t_core, cfg.sg
    sg_list = []
    k0 = 0
    while k0 < tcore:
        kn = min(sg, tcore - k0)
        sg_list.append((k0, kn))
        k0 += kn
    calls = []
    chunk_of = {}
    cpos = 0
    for si, (k0, kn) in enumerate(sg_list):
        for b in range(NBANK):
            c0 = cpos
            for ki in range(kn):
                k = k0 + ki
                for c in range(int(cstar[k, b])):
                    chunk_of[(k, b, c)] = cpos
                    cpos += 1
            if cpos > c0:
                calls.append((si, b, c0, cpos - c0))
    return sg_list, calls, chunk_of, cpos


def _fill_core_graph(plan, layout, core, cfg: Config):
    """Build IDX16 (wrapped), OFF and NSE arrays for one core, one graph."""
    sg_list, calls, chunk_of, n_chunks = layout
    cstar = plan["cstar"]
    idx = np.zeros((n_chunks, P), np.int16)
    off = np.full((n_chunks, P), 512.0, np.float32)
    nse = np.zeros((n_chunks, P), np.float32)
    for k in range(cfg.t_core):
        t = plan["core_tiles"][core, k]
        if t < 0:
            continue
        ts_src, ts_off, ts_nse, bc = plan["tile_edges"][t]
        bstart = np.zeros(NBANK + 1, np.int64)
        np.cumsum(bc, out=bstart[1:])
        for b in range(NBANK):
            nb = int(bc[b])
            cnum = int(cstar[k, b])
            if cnum == 0:
                continue
            nslot = cnum * P
            es = np.zeros(nslot, np.int64)
            eo = np.full(nslot, 512.0, np.float32)
            en = np.zeros(nslot, np.float32)
            es[:nb] = ts_src[bstart[b] : bstart[b + 1]] - b * cfg.bank_rows
            eo[:nb] = ts_off[bstart[b] : bstart[b + 1]]
            en[:nb] = ts_nse[bstart[b] : bstart[b + 1]]
            for c in range(cnum):
                ci = chunk_of[(k, b, c)]
                idx[ci] = es[c * P : (c + 1) * P]
                off[ci] = eo[c * P : (c + 1) * P]
                nse[ci] = en[c * P : (c + 1) * P]
    # wrap: flat slot i (within a call's range) -> [i%16, i//16], replicated
    # to 128 partitions. Build per call, concatenated along columns.
    ncols = n_chunks * P // 16
    idx_w = np.zeros((P, ncols), np.int16)
    for (si, b, c0, nch) in calls:
        flat = idx[c0 : c0 + nch].reshape(-1)  # ni slots
        blk = flat.reshape(-1, 16).T  # [16, ni/16]
        idx_w[:, c0 * 8 : (c0 + nch) * 8] = np.tile(blk, (8, 1))
    return idx_w, off.T.copy(), nse.T.copy()  # [P, n_chunks] f32


def preprocess(feats, W, b, prelu_a, src_pos, dst_pos, src_neg, dst_neg,
               cfg: Config):
    n, ncores = cfg.n_nodes, cfg.n_cores
    feats = np.asarray(feats, np.float32)
    W = np.asarray(W, np.float32)
    b = np.asarray(b, np.float32)
    prelu_a = np.asarray(prelu_a, np.float32)

    featsr = np.zeros((cfg.n_pad, D), BF16)  # row-major, padded, bf16
    featsr[:n] = feats.astype(BF16)

    plans, layouts = [], []
    for src, dst in ((src_pos, dst_pos), (src_neg, dst_neg)):
        src = np.asarray(src, np.int64)
        dst = np.asarray(dst, np.int64)
        deg_out = np.bincount(src, minlength=n).astype(np.float32)
        deg_in = np.bincount(dst, minlength=n).astype(np.float32)
        ns = np.where(deg_out > 0, 1.0 / np.sqrt(np.maximum(deg_out, 1.0)),
                      0.0).astype(np.float32)
        nd = np.where(deg_in > 0, 1.0 / np.sqrt(np.maximum(deg_in, 1.0)),
                      0.0).astype(np.float32)
        nse_edge = ns[src] * nd[dst]
        plan = _plan_graph(src, dst, nse_edge, cfg)
        plans.append(plan)
        layouts.append(_slot_layout(plan["cstar"], cfg))

    iota = np.tile(np.arange(P, dtype=np.float32), (P, 1)).astype(BF16)
    a_rep = np.full((P, 1), float(prelu_a.reshape(-1)[0]), np.float32)
    b_rep = np.tile(b.reshape(1, D), (P, 1)).astype(np.float32)

    in_maps = []
    for core in range(ncores):
        iw_p, off_p, nse_p = _fill_core_graph(plans[0], layouts[0], core, cfg)
        iw_n, off_n, nse_n = _fill_core_graph(plans[1], layouts[1], core, cfg)
        in_maps.append({
            "featsr": featsr,
            "w_in": W,
            "a_rep": a_rep,
            "b_rep": b_rep,
            "idx_in": np.concatenate([iw_p, iw_n], axis=1),
            "off_in": np.concatenate([off_p, off_n], axis=1),
            "nse_in": np.concatenate([nse_p, nse_n], axis=1),
            "iota_in": iota,
        })
    meta = {
        "layouts": layouts,
        "cstar": [plans[0]["cstar"], plans[1]["cstar"]],
        "use_bias": bool(np.any(b != 0.0)),
    }
    return in_maps, plans, meta


# --------------------------------------------------------------------------
# Device kernel builder
# --------------------------------------------------------------------------
def build_kernel(nc, tc, cfg: Config, meta):
    from contextlib import ExitStack

    import concourse.mybir as mybir

    f32 = mybir.dt.float32
    bf16 = mybir.dt.bfloat16
    i16 = mybir.dt.int16
    Alu = mybir.AluOpType
    Act = mybir.ActivationFunctionType

    tcore, npad = cfg.t_core, cfg.n_pad
    layouts = meta["layouts"]
    cstar = meta["cstar"]
    use_bias = meta["use_bias"]
    n_chunks = [layouts[g][3] for g in range(2)]
    ncols = [n_chunks[g] * P // 16 for g in range(2)]

    featsr = nc.dram_tensor("featsr", [npad, D], bf16,
                            kind="ExternalInput").ap()
    w_in = nc.dram_tensor("w_in", [P, D], f32, kind="ExternalInput").ap()
    a_rep = nc.dram_tensor("a_rep", [P, 1], f32, kind="ExternalInput").ap()
    b_rep = nc.dram_tensor("b_rep", [P, D], f32, kind="ExternalInput").ap()
    idx_in = nc.dram_tensor("idx_in", [P, sum(ncols)], i16,
                            kind="ExternalInput").ap()
    off_in = nc.dram_tensor("off_in", [P, sum(n_chunks)], f32,
                            kind="ExternalInput").ap()
    nse_in = nc.dram_tensor("nse_in", [P, sum(n_chunks)], f32,
                            kind="ExternalInput").ap()
    iota_in = nc.dram_tensor("iota_in", [P, P], bf16, kind="ExternalInput").ap()
    out = nc.dram_tensor("out", [2, tcore, P, D], bf16,
                         kind="ExternalOutput").ap()

    with ExitStack() as ctx:
        const = ctx.enter_context(tc.tile_pool(name="const", bufs=1))
        gpool = ctx.enter_context(tc.tile_pool(name="gpool", bufs=cfg.gbufs))
        ipool = ctx.enter_context(tc.tile_pool(name="ipool", bufs=cfg.ipbufs))
        ohpool = ctx.enter_context(tc.tile_pool(name="ohpool", bufs=6))
        aggpool = ctx.enter_context(tc.tile_pool(name="aggpool", bufs=4))
        tpool = ctx.enter_context(tc.tile_pool(name="tpool", bufs=4))
        spool = ctx.enter_context(tc.tile_pool(name="spool", bufs=3))
        ppool = ctx.enter_context(tc.tile_pool(name="ppool", bufs=5,
                                               space="PSUM"))
        hpool = ctx.enter_context(tc.tile_pool(name="hpool", bufs=3,
                                               space="PSUM"))

        # ---- constants ----
        w_sb = const.tile([P, D], bf16)
        nc.gpsimd.dma_start(out=w_sb[:], in_=w_in)  # f32 -> bf16 cast DMA
        iota_sb = const.tile([P, P], bf16)
        nc.sync.dma_start(out=iota_sb[:], in_=iota_in)
        a_sb = const.tile([P, 1], f32)
        nc.sync.dma_start(out=a_sb[:], in_=a_rep)
        if use_bias:
            b_sb = const.tile([P, D], f32)
            nc.sync.dma_start(out=b_sb[:], in_=b_rep)

        # ---- gather + weighted one-hot segment-sum + @W + prelu ----
        col_base = [0, ncols[0]]          # idx column offset per graph
        chk_base = [0, n_chunks[0]]       # off/nse column offset per graph
        cbs_all = []
        for g in range(2):
            calls_by_sg = {}
            for (si, b, c0, nch) in layouts[g][1]:
                calls_by_sg.setdefault(si, []).append((b, c0, nch))
            cbs_all.append(calls_by_sg)
        # interleave the two graphs' supergroups so one graph's gathers fill
        # DMA while the other's PSUM chain drains
        jobs = []
        for si in range(max(len(layouts[0][0]), len(layouts[1][0]))):
            for g in range(2):
                if si < len(layouts[g][0]):
                    jobs.append((g, si))
        ecnt = 0
        for (g, si) in jobs:
            sg_list, calls, chunk_of, _ = layouts[g]
            cs = cstar[g]
            calls_by_sg = cbs_all[g]
            (k0, kn) = sg_list[si]
            sg_chunks = sum(int(cs[k0 + ki, b]) for ki in range(kn)
                            for b in range(NBANK))
            c0_sg = chunk_of[(k0, 0, 0)]
            gt = gpool.tile([P, sg_chunks, D], bf16, tag="gather")
            it = ipool.tile([P, sg_chunks * 8], i16, tag="gidx")
            nc.sync.dma_start(
                out=it[:],
                in_=idx_in[:, col_base[g] + c0_sg * 8 :
                           col_base[g] + (c0_sg + sg_chunks) * 8])
            ot = ipool.tile([P, sg_chunks], f32, tag="goff")
            nc.sync.dma_start(
                out=ot[:],
                in_=off_in[:, chk_base[g] + c0_sg :
                           chk_base[g] + c0_sg + sg_chunks])
            et = ipool.tile([P, sg_chunks], f32, tag="gnse")
            nc.sync.dma_start(
                out=et[:],
                in_=nse_in[:, chk_base[g] + c0_sg :
                           chk_base[g] + c0_sg + sg_chunks])
            for (b, c0, nch) in calls_by_sg[si]:
                lo = c0 - c0_sg
                bank_rows = min(cfg.bank_rows, npad - b * cfg.bank_rows)
                nc.gpsimd.dma_gather(
                    out_ap=gt[:, lo : lo + nch, :],
                    in_ap=featsr[b * cfg.bank_rows :
                                 b * cfg.bank_rows + bank_rows, :],
                    idxs_ap=it[:, lo * 8 : (lo + nch) * 8],
                    num_idxs=nch * P, num_idxs_reg=nch * P,
                    elem_size=D, single_packet=False)
            stg = spool.tile([P, kn, D], bf16, tag="stg")
            for ki in range(kn):
                k = k0 + ki
                nonzero = [(b, c) for b in range(NBANK)
                           for c in range(int(cs[k, b]))]
                psT = ppool.tile([P, D], f32)  # aggT: [feat, dstoff]
                for j, (b, c) in enumerate(nonzero):
                    ci = chunk_of[(k, b, c)]
                    lo = ci - c0_sg
                    oh = ohpool.tile([P, P], bf16)
                    eng = nc.vector
                    if cfg.oh_gpsimd_mod and (ci % cfg.oh_gpsimd_mod == 0):
                        eng = nc.gpsimd
                    eng.tensor_scalar(
                        out=oh[:], in0=iota_sb[:],
                        scalar1=ot[:, lo : lo + 1],
                        scalar2=et[:, lo : lo + 1],
                        op0=Alu.is_equal, op1=Alu.mult)
                    nc.tensor.matmul(
                        out=psT[:], lhsT=gt[:, lo, :], rhs=oh[:],
                        start=(j == 0), stop=(j == len(nonzero) - 1))
                # cast aggT -> SBUF bf16 (alternate engines), then @W
                aggsb = aggpool.tile([P, D], bf16, tag="aggsb")
                if ecnt % 2 == 0:
                    nc.scalar.activation(out=aggsb[:], in_=psT[:],
                                         func=Act.Copy)
                else:
                    nc.vector.tensor_copy(out=aggsb[:], in_=psT[:])
                ecnt += 1
                hps = hpool.tile([P, D], f32)
                nc.tensor.matmul(out=hps[:], lhsT=aggsb[:], rhs=w_sb[:],
                                 start=True, stop=True)
                if use_bias:
                    hb2 = tpool.tile([P, D], f32, tag="hb2")
                    nc.vector.tensor_tensor(out=hb2[:], in0=hps[:],
                                            in1=b_sb[:], op=Alu.add)
                    neg = tpool.tile([P, D], f32, tag="neg")
                    nc.vector.tensor_scalar(
                        out=neg[:], in0=hb2[:], scalar1=0.0,
                        scalar2=a_sb[:, :1], op0=Alu.min, op1=Alu.mult)
                    pos = tpool.tile([P, D], f32, tag="pos")
                    nc.vector.tensor_scalar(
                        out=pos[:], in0=hb2[:], scalar1=0.0,
                        scalar2=None, op0=Alu.max)
                    nc.vector.tensor_tensor(out=stg[:, ki, :], in0=neg[:],
                                            in1=pos[:], op=Alu.add)
                elif cfg.act_prelu:
                    nc.scalar.activation(
                        out=stg[:, ki, :], in_=hps[:], func=Act.Prelu,
                        alpha=a_sb[:, :1])
                else:
                    neg = tpool.tile([P, D], f32, tag="neg")
                    nc.vector.tensor_scalar(
                        out=neg[:], in0=hps[:], scalar1=0.0,
                        scalar2=a_sb[:, :1], op0=Alu.min, op1=Alu.mult)
                    pos = tpool.tile([P, D], f32, tag="pos")
                    nc.vector.tensor_scalar(
                        out=pos[:], in0=hps[:], scalar1=0.0,
                        scalar2=None, op0=Alu.max)
                    nc.vector.tensor_tensor(out=stg[:, ki, :], in0=neg[:],
                                            in1=pos[:], op=Alu.add)
            nc.sync.dma_start(
                out=out[g, k0 : k0 + kn, :, :].rearrange("k p d -> p k d"),
                in_=stg[:])
    return out


# --------------------------------------------------------------------------
# Driver
# --------------------------------------------------------------------------
def _build_program(cfg: Config, meta):
    import concourse.bacc as bacc
    import concourse.tile as tile

    nc = bacc.Bacc("TRN2", target_bir_lowering=False, debug=False,
                   enable_asserts=False, num_devices=cfg.n_cores)
    with tile.TileContext(nc) as tc:
        build_kernel(nc, tc, cfg, meta)
    nc.compile()
    return nc


def _unscramble(results, plans, cfg: Config):
    n = cfg.n_nodes
    full = np.zeros((2, n, D), np.float32)
    for g in range(2):
        ct_all = plans[g]["core_tiles"]
        for core in range(cfg.n_cores):
            oc = np.asarray(results[core]["out"], dtype=np.float32)
            for k in range(cfg.t_core):
                t = int(ct_all[core, k])
                if t < 0:
                    continue
                r0 = t * P
                r1 = min(r0 + P, n)
                full[g, r0:r1] = oc[g, k, : r1 - r0, :]
    return full


_PROGRAM_CACHE = {}


def run(inputs, cfg: Config, trace=False):
    from concourse.bass_utils import run_bass_kernel_spmd

    in_maps, plans, meta = preprocess(
        inputs["feats"], inputs["W"], inputs["b"], inputs["prelu_a"],
        inputs["src_pos"], inputs["dst_pos"],
        inputs["src_neg"], inputs["dst_neg"], cfg)

    key = (cfg.n_nodes, cfg.n_cores, cfg.sg,
           cfg.act_prelu, cfg.oh_gpsimd_mod, cfg.gbufs,
           meta["cstar"][0].tobytes(), meta["cstar"][1].tobytes(),
           meta["use_bias"])
    nc = _PROGRAM_CACHE.get(key)
    if nc is None:
        nc = _build_program(cfg, meta)
        _PROGRAM_CACHE[key] = nc

    kwargs = {}
    if trace:
        kwargs = dict(trace=True, tmpdir=tempfile.mkdtemp(prefix="bgc_trace_"))
    res = run_bass_kernel_spmd(nc, in_maps, core_ids=list(range(cfg.n_cores)),
                               **kwargs)
    full = _unscramble(res.results, plans, cfg)
    return full, res


def kernel(**inputs) -> np.ndarray:
    cfg = Config()
    full, _ = run(inputs, cfg)
    return full
